# revision 1
# baseline (speedup 1.0000x reference)
"""DiffFDN Trainium2 kernel, v3: DRAM history + indirect gathers.

Per core (4 items): the 48000-step FDN scan becomes 94 blocks of
[64x68]^T @ [64x500] (float32r). History lives in DRAM as one
[68, TPAD] tensor (rows 0-63 per-(line,item) nxt series, rows 64-67 the
y output series). Per block: one PSUM->SBUF copy, one staged HWDGE
write to DRAM, one SWDGE *indirect* gather (per-row flat offsets) that
assembles the 16 time-shifted line reads in a single instruction.

The gather's in_ AP is the column-sliced prefix H[:, 0:PAD+n0-500] so
the Tile dependency tracker sees the true dependency (the write from
block b-2), keeping a 2-deep software pipeline; offsets are view-flat
element indices r*X_b + (PAD + n0 - d_i).
"""

import numpy as np

SR = 48000
IR_LEN = 48000
DELAYS = [1009, 1123, 1231, 1321, 1433, 1543, 1657, 1777, 1879, 1987,
          2081, 2179, 2287, 2383, 2503, 2617]
N = 16
FEAT = 256
BATCH = 32
NCORES = 8
IPC = BATCH // NCORES
L = 500
PAD = 2620                     # zero padding before t=0 (>= max delay)
TPAD = PAD + IR_LEN + 500
NBLK = IR_LEN // L             # 96; blocks 0,1 skipped (all-zero)
M_OUT = IPC * N + IPC          # 68

_BUILT = None
K_DEP = 2          # gather only covers blocks <= b-K_DEP (pipeline depth)


def _patch_list():
    """Pieces of each line's read window sourced from blocks > b-K_DEP.

    These are patched into S from the SBUF stage tiles (the DRAM gather
    raced/skipped those regions). Returns (line, rel_blk, src_col,
    dst_col, length) tuples; rel_blk is source block minus b.
    """
    out = []
    for i in range(N):
        d = DELAYS[i]
        lo, hi = -d, -d + L
        a = lo
        while a < hi:
            e = min(hi, (a // L + 1) * L)
            rel = a // L
            if rel >= -(K_DEP - 1):
                out.append((i, rel, a - rel * L, a - lo, e - a))
            a = e
    return out


def _expm64(M):
    M = M.astype(np.float64)
    nrm = np.linalg.norm(M, ord=np.inf)
    k = max(0, int(np.ceil(np.log2(max(nrm, 1e-30)))) + 2)
    Ms = M / (2.0 ** k)
    E = np.eye(M.shape[0]) + Ms
    term = Ms.copy()
    for i in range(2, 18):
        term = term @ Ms / i
        E = E + term
    for _ in range(k):
        E = E @ E
    return E


def _prologue(x, WA, bA, WB, bB, WC, bC):
    x = np.asarray(x, np.float32)
    feat = x.mean(axis=1)
    A = np.tanh(feat @ np.asarray(WA).T + bA).reshape(-1, N, N)
    Bv = np.tanh(feat @ np.asarray(WB).T + bB)
    Cv = np.tanh(feat @ np.asarray(WC).T + bC)
    S = np.triu(A, 1)
    S = S - np.swapaxes(S, -1, -2)
    g = 10.0 ** (-3.0 / SR)
    G = g ** np.asarray(DELAYS, np.float64)
    A_g = np.stack([_expm64(S[b]) for b in range(S.shape[0])])
    A_g = (A_g * G[None, None, :]).astype(np.float32)
    return A_g, Bv.astype(np.float32), Cv.astype(np.float32)


def _core_inputs(A_g4, Bv4, Cv4):
    lhsT = np.zeros((IPC * N, M_OUT), np.float32)
    bv = np.zeros((IPC * N, 1), np.float32)
    for j in range(IPC):
        for i in range(N):
            r = 4 * i + j
            for ip in range(N):
                lhsT[r, 4 * ip + j] = A_g4[j, ip, i]
            lhsT[r, IPC * N + j] = Cv4[j, i]
            bv[r, 0] = Bv4[j, i]
    return lhsT, bv


OFFS_PHYSICAL = True


def _offsets():
    """offs[r, b-2] = flat gather offset for row r, sub-block b.

    OFFS_PHYSICAL: offsets are element offsets into the physical tensor
    (row stride TPAD) -- what the HW descriptor generator uses. CoreSim
    instead flattens the sliced view (row stride X_b).
    """
    offs = np.zeros((IPC * N, NBLK - 2), np.uint32)
    for b in range(2, NBLK):
        n0 = L * b
        Xb = TPAD if OFFS_PHYSICAL else (PAD + n0 - L)
        for i in range(N):
            for j in range(IPC):
                r = 4 * i + j
                offs[r, b - 2] = r * Xb + (PAD + n0 - DELAYS[i])
    return offs


def _build():
    global _BUILT
    if _BUILT is not None:
        return _BUILT
    import concourse.bacc as bacc
    import concourse.bass as bass
    import concourse.mybir as mybir
    import concourse.tile as tile

    fp32 = mybir.dt.float32
    f32r = mybir.dt.float32r
    u32 = mybir.dt.uint32
    nc = bacc.Bacc("TRN2", target_bir_lowering=False, debug=False)
    lhsT_d = nc.dram_tensor("lhsT", [IPC * N, M_OUT], f32r, kind="ExternalInput")
    bv_d = nc.dram_tensor("bv", [IPC * N, 1], f32r, kind="ExternalInput")
    offs_d = nc.dram_tensor("offs", [IPC * N, NBLK - 2], u32, kind="ExternalInput")
    patches = _patch_list()
    npat = len(patches)
    pmask_d = None
    if npat:
        pmask_d = nc.dram_tensor(
            "pmask", [IPC * N, npat], mybir.dt.uint8, kind="ExternalInput")
    y_d = nc.dram_tensor("y", [IPC, IR_LEN], f32r, kind="ExternalOutput")
    h_d = nc.dram_tensor("hist", [M_OUT, TPAD], f32r)

    with tile.TileContext(nc) as tc:
        with tc.tile_pool(name="const", bufs=1) as cpool, \
             tc.tile_pool(name="init", bufs=1) as ipool, \
             tc.tile_pool(name="sg", bufs=8) as spool, \
             tc.tile_pool(name="st", bufs=10) as tpool, \
             tc.tile_pool(name="ps", bufs=8, space="PSUM") as ppool, \
             tc.tile_pool(name="yb", bufs=2) as ypool:
            lhsT = cpool.tile([IPC * N, M_OUT], f32r)
            nc.sync.dma_start(lhsT[:, :], lhsT_d[:, :])
            offs = cpool.tile([IPC * N, NBLK - 2], u32)
            nc.sync.dma_start(offs[:, :], offs_d[:, :])
            pmask = None
            if npat:
                pmask = cpool.tile([IPC * N, npat], mybir.dt.uint8)
                nc.sync.dma_start(pmask[:, :], pmask_d[:, :])

            # zero-init history cols [0, PAD+1000) incl. y rows; Bv impulse
            # lands at col PAD (time 0) via the same staged image.
            z = ipool.tile([M_OUT, PAD + 2 * L], fp32)
            half = (PAD + 2 * L) // 2
            nc.vector.memset(z[:, 0:half], 0.0)
            nc.gpsimd.memset(z[:, half:], 0.0)
            nc.sync.dma_start(z[0:IPC * N, PAD:PAD + 1].bitcast(f32r), bv_d[:, :])
            nc.scalar.dma_start(
                h_d[:, 0:PAD + 2 * L].bitcast(fp32), z[:, :])

            stages = {}  # b -> (tile, col0) holding that block's nxt in SBUF
            stages[-1] = (z, PAD - L)   # negative time: zeros
            stages[0] = (z, PAD)
            stages[1] = (z, PAD + L)
            for b in range(2, NBLK):
                n0 = L * b
                Xb = PAD + n0 - (K_DEP - 1) * L
                S = spool.tile([IPC * N, L], f32r)
                nc.gpsimd.indirect_dma_start(
                    out=S[:, :], out_offset=None,
                    in_=h_d[0:IPC * N, 0:Xb],
                    in_offset=bass.IndirectOffsetOnAxis(
                        ap=offs[:, b - 2:b - 1], axis=1),
                )
                # patch recent-sourced pieces of S from the SBUF stages
                # (the DRAM gather raced/skipped those regions). Engines
                # require 32-aligned partition bases, so each patch runs
                # base-0 over all rows with a per-line row mask.
                for k, (i, rel, sc, dc, ln) in enumerate(patches):
                    ptile, pcol = stages[b + rel]
                    src = ptile[0:IPC * N, pcol + sc:pcol + sc + ln]
                    if src.dtype != f32r:
                        src = src.bitcast(f32r)
                    nc.vector.copy_predicated(
                        S[:, dc:dc + ln],
                        pmask[:, k:k + 1].to_broadcast([IPC * N, ln]),
                        src,
                    )
                ps = ppool.tile([M_OUT, L], fp32)
                nc.tensor.matmul(ps[:, :], lhsT[:, :], S[:, :],
                                 start=True, stop=True)
                stage = tpool.tile([M_OUT, L], f32r)
                stages[b] = (stage, 0)
                if b % 2 == 0:
                    nc.vector.tensor_copy(stage[:, :], ps[:, :])
                else:
                    nc.scalar.copy(stage[:, :], ps[:, :])
                weng = nc.sync if b % 2 == 0 else nc.scalar
                weng.dma_start(h_d[:, PAD + n0:PAD + n0 + L], stage[:, :])

                # y extraction (hist rows 64..67 -> y, bounced via SBUF),
                # interleaved: chunk k is final once block 24*(k+1) has
                # been written, so it overlaps the remaining compute
                # instead of trailing the last block.
                CH = 12000
                if b >= 25 and (b - 25) % 24 == 0 and (k := (b - 25) // 24) < 3:
                    yb = ypool.tile([IPC, CH], f32r)
                    nc.scalar.dma_start(
                        yb[:, :],
                        h_d[IPC * N:M_OUT, PAD + k * CH:PAD + (k + 1) * CH])
                    nc.scalar.dma_start(y_d[:, k * CH:(k + 1) * CH], yb[:, :])
                # last two blocks: ship y straight from the SBUF stage so
                # the kernel tail doesn't wait on their DRAM writes
                if b >= NBLK - 2:
                    nc.sync.dma_start(
                        y_d[:, n0:n0 + L], stage[IPC * N:M_OUT, :])

            # remaining y span [3*CH, last two blocks) after the loop;
            # reads only blocks <= NBLK-3 so it overlaps the loop tail
            yb = ypool.tile([IPC, CH], f32r)
            span = IR_LEN - L * 2 - 3 * CH
            nc.scalar.dma_start(
                yb[:, 0:span],
                h_d[IPC * N:M_OUT, PAD + 3 * CH:PAD + 3 * CH + span])
            nc.scalar.dma_start(y_d[:, 3 * CH:3 * CH + span], yb[:, 0:span])
    nc.compile()
    _BUILT = nc
    return nc


def _pmask():
    pl = _patch_list()
    pm = np.zeros((IPC * N, len(pl)), np.uint8)
    for k, (i, _, _, _, _) in enumerate(pl):
        pm[4 * i:4 * i + 4, k] = 1
    return pm


def kernel(x, WA, bA, WB, bB, WC, bC):
    from concourse import bass_utils

    A_g, Bv, Cv = _prologue(x, WA, bA, WB, bB, WC, bC)
    offs = _offsets()
    pm = _pmask()
    in_maps = []
    for k in range(NCORES):
        sl = slice(k * IPC, (k + 1) * IPC)
        lhsT, bv = _core_inputs(A_g[sl], Bv[sl], Cv[sl])
        m = {"lhsT": lhsT, "bv": bv, "offs": offs}
        if pm.shape[1]:
            m["pmask"] = pm
        in_maps.append(m)

    nc = _build()
    res = bass_utils.run_bass_kernel_spmd(nc, in_maps, core_ids=list(range(NCORES)))
    y = np.concatenate([res.results[k]["y"] for k in range(NCORES)], axis=0)
    return y[:, None, :].astype(np.float32)



# revision 5
# speedup vs baseline: 1.3098x; 1.3098x over previous
"""DiffFDN Trainium2 kernel, v5: SBUF ring fast-path + slack DMA gather.

Per core (4 items), the 48000-step FDN scan runs as 94 blocks of L=500
timesteps. The serial dependency (min delay 1009) only couples block b
to blocks <= b-2, and the per-block work is one [128x128]^T @ [128x500]
matmul pair.

Key structure (vs the old all-DMA pipeline):
- Deep lines (d >= 1543, lines 5-15): windows come from blocks <= b-3,
  fetched by ONE SWDGE indirect gather per block whose declared in_ AP
  prefix ends at the b-3 frontier -- so it waits only on write(b-3),
  giving the DMA round trip a full extra link of slack off the critical
  chain.
- Shallow lines (d < 1500, lines 0-4): their windows touch block b-2,
  too fresh for DRAM. They are assembled into a separate C tile from an
  SBUF stage ring (drain targets) by 4 plain DVE copies at quadrant
  partition bases (0/32/64/96; engines require 32-aligned partition
  bases) plus one predicated copy for line 4. The serial chain is then
  drain(b-2) -> movers(b) -> mm_C(b): engine ops only, no DMA.
- Row layout spreads the 64 (line,item) rows across 128 partitions so
  each shallow line sits at its own 32-aligned base; lhsT rows for
  unused partitions are zero, which makes the extra partitions free in
  the matmul (cost scales only with the 500-wide free dim).
"""

import numpy as np

SR = 48000
IR_LEN = 48000
DELAYS = [1009, 1123, 1231, 1321, 1433, 1543, 1657, 1777, 1879, 1987,
          2081, 2179, 2287, 2383, 2503, 2617]
N = 16
FEAT = 256
BATCH = 32
NCORES = 8
IPC = BATCH // NCORES
L = 500
PAD = 2620                     # zero padding before t=0 (>= max delay)
TPAD = PAD + IR_LEN + 500
NBLK = IR_LEN // L             # 96; blocks 0,1 are part of the init image
NROW = 128
RING = 4                       # stage-ring slots (plus one mirror slot)

# partition base of each line's 4 item-rows; shallow lines 0-3 take the
# 32-aligned quadrant bases so plain DVE copies can address them.
PBASE = [0, 32, 64, 96, 4, 8, 12, 16, 20, 24, 28, 36, 40, 44, 48, 52]
YBASE = 56                     # y output rows (4)
NCRIT = 5                      # lines 0-4: d < 3L, SBUF fast path
DEEP = list(range(NCRIT, N))   # lines via DRAM gather

_BUILT = None


def _expm64(M):
    M = M.astype(np.float64)
    nrm = np.linalg.norm(M, ord=np.inf)
    k = max(0, int(np.ceil(np.log2(max(nrm, 1e-30)))) + 2)
    Ms = M / (2.0 ** k)
    E = np.eye(M.shape[0]) + Ms
    term = Ms.copy()
    for i in range(2, 18):
        term = term @ Ms / i
        E = E + term
    for _ in range(k):
        E = E @ E
    return E


def _prologue(x, WA, bA, WB, bB, WC, bC):
    x = np.asarray(x, np.float32)
    feat = x.mean(axis=1)
    A = np.tanh(feat @ np.asarray(WA).T + bA).reshape(-1, N, N)
    Bv = np.tanh(feat @ np.asarray(WB).T + bB)
    Cv = np.tanh(feat @ np.asarray(WC).T + bC)
    S = np.triu(A, 1)
    S = S - np.swapaxes(S, -1, -2)
    g = 10.0 ** (-3.0 / SR)
    G = g ** np.asarray(DELAYS, np.float64)
    A_g = np.stack([_expm64(S[b]) for b in range(S.shape[0])])
    A_g = (A_g * G[None, None, :]).astype(np.float32)
    return A_g, Bv.astype(np.float32), Cv.astype(np.float32)


def _core_inputs(A_g4, Bv4, Cv4):
    """Per-core constant tensors in the 128-partition spread layout.

    lhsT_C covers the shallow lines' contribution (rhs = C tile),
    lhsT_S the deep lines' (rhs = gathered S tile). lhsT[k, m]:
    k = rhs partition (source line row), m = out partition.
    """
    lhsT_C = np.zeros((NROW, NROW), np.float32)
    lhsT_4 = np.zeros((NROW, NROW), np.float32)
    lhsT_S = np.zeros((NROW, NROW), np.float32)
    bv = np.zeros((NROW, 1), np.float32)
    for j in range(IPC):
        for i in range(N):
            r = PBASE[i] + j
            dst = lhsT_C if i < 4 else (lhsT_4 if i == 4 else lhsT_S)
            for ip in range(N):
                dst[r, PBASE[ip] + j] = A_g4[j, ip, i]
            dst[r, YBASE + j] = Cv4[j, i]
            bv[r, 0] = Bv4[j, i]
    return lhsT_C, lhsT_4, lhsT_S, bv


def _offsets():
    """offs[r, b-2] = physical flat gather offset for row r, block b.

    Deep-line rows read their [n0-d_i, n0-d_i+500) window; all other
    rows point at their own row's column 0 (zero-init region) so the
    gather stays a single 128-descriptor instruction.
    """
    offs = np.zeros((NROW, NBLK - 2), np.uint32)
    for r in range(NROW):
        offs[r, :] = r * TPAD
    for b in range(2, NBLK):
        n0 = L * b
        for i in DEEP:
            for j in range(IPC):
                r = PBASE[i] + j
                offs[r, b - 2] = r * TPAD + (PAD + n0 - DELAYS[i])
    return offs


def _make_in_maps(A_g, Bv, Cv):
    offs = _offsets()
    in_maps = []
    for k in range(NCORES):
        sl = slice(k * IPC, (k + 1) * IPC)
        lhsT_C, lhsT_4, lhsT_S, bv = _core_inputs(A_g[sl], Bv[sl], Cv[sl])
        in_maps.append({"lhsTC": lhsT_C, "lhsT4": lhsT_4, "lhsTS": lhsT_S,
                        "bv": bv, "offs": offs})
    return in_maps


def _build():
    global _BUILT
    if _BUILT is not None:
        return _BUILT
    import concourse.bacc as bacc
    import concourse.bass as bass
    import concourse.mybir as mybir
    import concourse.tile as tile

    fp32 = mybir.dt.float32
    f32r = mybir.dt.float32r
    u32 = mybir.dt.uint32
    u8 = mybir.dt.uint8
    nc = bacc.Bacc("TRN2", target_bir_lowering=False, debug=False)
    lhsTC_d = nc.dram_tensor("lhsTC", [NROW, NROW], f32r, kind="ExternalInput")
    lhsT4_d = nc.dram_tensor("lhsT4", [NROW, NROW], f32r, kind="ExternalInput")
    lhsTS_d = nc.dram_tensor("lhsTS", [NROW, NROW], f32r, kind="ExternalInput")
    bv_d = nc.dram_tensor("bv", [NROW, 1], f32r, kind="ExternalInput")
    offs_d = nc.dram_tensor("offs", [NROW, NBLK - 2], u32, kind="ExternalInput")
    y_d = nc.dram_tensor("y", [IPC, IR_LEN], f32r, kind="ExternalOutput")
    h_d = nc.dram_tensor("hist", [NROW, TPAD], f32r)

    RW = (RING + 1) * L        # ring cols incl. mirror slot
    SH4 = 3 * L - DELAYS[4]    # line-4 window offset in the ring view

    with tile.TileContext(nc) as tc:
        with tc.tile_pool(name="const", bufs=1) as cpool, \
             tc.tile_pool(name="init", bufs=1) as ipool, \
             tc.tile_pool(name="ring", bufs=1) as rpool, \
             tc.tile_pool(name="sg", bufs=6) as spool, \
             tc.tile_pool(name="cc", bufs=3) as ccpool, \
             tc.tile_pool(name="ps", bufs=8, space="PSUM") as ppool, \
             tc.tile_pool(name="yb", bufs=2) as ypool:
            lhsTC = cpool.tile([NROW, NROW], f32r)
            nc.sync.dma_start(lhsTC[:, :], lhsTC_d[:, :])
            lhsT4 = cpool.tile([NROW, NROW], f32r)
            nc.sync.dma_start(lhsT4[:, :], lhsT4_d[:, :])
            lhsTS = cpool.tile([NROW, NROW], f32r)
            nc.sync.dma_start(lhsTS[:, :], lhsTS_d[:, :])
            offs = cpool.tile([NROW, NBLK - 2], u32)
            nc.sync.dma_start(offs[:, :], offs_d[:, :])

            # DRAM history init: zeros over [0, PAD+2L) with the Bv
            # impulse at col PAD (time 0).
            z = ipool.tile([NROW, PAD + 2 * L], fp32)
            half = (PAD + 2 * L) // 2
            nc.vector.memset(z[:, 0:half], 0.0)
            nc.gpsimd.memset(z[:, half:], 0.0)
            nc.sync.dma_start(z[:, PAD:PAD + 1].bitcast(f32r), bv_d[:, :])
            nc.scalar.dma_start(h_d[:, 0:PAD + 2 * L].bitcast(fp32), z[:, :])

            # Stage ring: slot s holds block b's outputs for b % RING == s;
            # cols [RING*L, RING*L+L) mirror slot 0 so the 2-slot read
            # window stays contiguous across the wrap. Preload: slot 3 :=
            # block -1 (zeros), slot 0 := block 0 (Bv at col 0), slot 1 :=
            # block 1 (zeros).
            ring = rpool.tile([NROW, RW], f32r)
            nc.sync.dma_start(ring[:, :], h_d[:, 0:RW])
            nc.sync.dma_start(ring[:, 0:1], bv_d[:, :])
            nc.vector.tensor_copy(ring[:, RING * L:RING * L + 1],
                                  ring[:, 0:1])
            for cb in range(3):
                ct = ccpool.tile([NROW, L], f32r)
                nc.scalar.dma_start(ct[:, :], h_d[:, 0:L])

            for b in range(2, NBLK):
                n0 = L * b
                Xb = PAD + n0 - 2 * L   # b-3 frontier (declared coverage)
                S = spool.tile([NROW, L], f32r)
                nc.gpsimd.indirect_dma_start(
                    out=S[:, :], out_offset=None,
                    in_=h_d[0:NROW, 0:Xb],
                    in_offset=bass.IndirectOffsetOnAxis(
                        ap=offs[:, b - 2:b - 1], axis=1),
                )

                # ring view holding blocks b-3, b-2 contiguously
                s3 = (b - 3) % RING
                w0 = s3 * L
                rw = ring[:, w0:w0 + 2 * L]

                # shallow-line movers: C[line rows, :] <- ring window at
                # the line's shift (cols 3L-d_i .. 3L-d_i+500)
                C = ccpool.tile([NROW, L], f32r)
                for i in range(4):
                    q = PBASE[i]
                    sh = 3 * L - DELAYS[i]
                    nc.vector.tensor_copy(C[q:q + IPC, :],
                                          rw[q:q + IPC, sh:sh + L])
                ps = ppool.tile([NROW, L], fp32)
                nc.tensor.matmul(ps[:, :], lhsTC[:, :], C[:, :],
                                 start=True, stop=False)
                nc.tensor.matmul(ps[:, :], lhsT4[:, :],
                                 rw[:, SH4:SH4 + L],
                                 start=False, stop=False)
                nc.tensor.matmul(ps[:, :], lhsTS[:, :], S[:, :],
                                 start=False, stop=True)

                # drain to ring slot (ACT), mirror slot 0 for the wrap
                slot = b % RING
                dst = ring[:, slot * L:slot * L + L]
                nc.scalar.copy(dst, ps[:, :])
                if slot == 0:
                    nc.vector.tensor_copy(
                        ring[:, RING * L:RING * L + L], ring[:, 0:L])

                weng = nc.sync if b % 2 == 0 else nc.scalar
                weng.dma_start(h_d[:, PAD + n0:PAD + n0 + L], dst)

                # y extraction, interleaved with compute (as in v3)
                CH = 12000
                if b >= 25 and (b - 25) % 24 == 0 and (k := (b - 25) // 24) < 3:
                    yb = ypool.tile([IPC, CH], f32r)
                    nc.scalar.dma_start(
                        yb[:, :],
                        h_d[YBASE:YBASE + IPC, PAD + k * CH:PAD + (k + 1) * CH])
                    nc.scalar.dma_start(y_d[:, k * CH:(k + 1) * CH], yb[:, :])
                if b >= NBLK - 2:
                    nc.sync.dma_start(
                        y_d[:, n0:n0 + L], dst[YBASE:YBASE + IPC, :])

            yb = ypool.tile([IPC, CH], f32r)
            span = IR_LEN - L * 2 - 3 * CH
            nc.scalar.dma_start(
                yb[:, 0:span],
                h_d[YBASE:YBASE + IPC, PAD + 3 * CH:PAD + 3 * CH + span])
            nc.scalar.dma_start(y_d[:, 3 * CH:3 * CH + span], yb[:, 0:span])
    nc.compile()
    _BUILT = nc
    return nc


def kernel(x, WA, bA, WB, bB, WC, bC):
    from concourse import bass_utils

    A_g, Bv, Cv = _prologue(x, WA, bA, WB, bB, WC, bC)
    in_maps = _make_in_maps(A_g, Bv, Cv)
    nc = _build()
    res = bass_utils.run_bass_kernel_spmd(nc, in_maps, core_ids=list(range(NCORES)))
    y = np.concatenate([res.results[k]["y"] for k in range(NCORES)], axis=0)
    return y[:, None, :].astype(np.float32)


# revision 7
# speedup vs baseline: 1.3302x; 1.0156x over previous
"""DiffFDN Trainium2 kernel, v5: SBUF ring fast-path + slack DMA gather.

Per core (4 items), the 48000-step FDN scan runs as 94 blocks of L=500
timesteps. The serial dependency (min delay 1009) only couples block b
to blocks <= b-2, and the per-block work is one [128x128]^T @ [128x500]
matmul pair.

Key structure (vs the old all-DMA pipeline):
- Deep lines (d >= 1543, lines 5-15): windows come from blocks <= b-3,
  fetched by ONE SWDGE indirect gather per block whose declared in_ AP
  prefix ends at the b-3 frontier -- so it waits only on write(b-3),
  giving the DMA round trip a full extra link of slack off the critical
  chain.
- Shallow lines (d < 1500, lines 0-4): their windows touch block b-2,
  too fresh for DRAM. They are assembled into a separate C tile from an
  SBUF stage ring (drain targets) by 4 plain DVE copies at quadrant
  partition bases (0/32/64/96; engines require 32-aligned partition
  bases) plus one predicated copy for line 4. The serial chain is then
  drain(b-2) -> movers(b) -> mm_C(b): engine ops only, no DMA.
- Row layout spreads the 64 (line,item) rows across 128 partitions so
  each shallow line sits at its own 32-aligned base; lhsT rows for
  unused partitions are zero, which makes the extra partitions free in
  the matmul (cost scales only with the 500-wide free dim).
"""

import numpy as np

SR = 48000
IR_LEN = 48000
DELAYS = [1009, 1123, 1231, 1321, 1433, 1543, 1657, 1777, 1879, 1987,
          2081, 2179, 2287, 2383, 2503, 2617]
N = 16
FEAT = 256
BATCH = 32
NCORES = 8
IPC = BATCH // NCORES
L = 500
PAD = 2620                     # zero padding before t=0 (>= max delay)
TPAD = PAD + IR_LEN + 500
NBLK = IR_LEN // L             # 96; blocks 0,1 are part of the init image
NROW = 128
RING = 4                       # stage-ring slots (plus one mirror slot)

# partition base of each line's 4 item-rows; shallow lines 0-3 take the
# 32-aligned quadrant bases so plain DVE copies can address them.
PBASE = [0, 32, 64, 96, 4, 8, 12, 16, 20, 24, 28, 36, 40, 44, 48, 52]
YBASE = 56                     # y output rows (4)
NCRIT = 5                      # lines 0-4: d < 3L, SBUF fast path
DEEP = list(range(NCRIT, N))   # lines via DRAM gather

_BUILT = None


def _expm64(M):
    M = M.astype(np.float64)
    nrm = np.linalg.norm(M, ord=np.inf)
    k = max(0, int(np.ceil(np.log2(max(nrm, 1e-30)))) + 2)
    Ms = M / (2.0 ** k)
    E = np.eye(M.shape[0]) + Ms
    term = Ms.copy()
    for i in range(2, 18):
        term = term @ Ms / i
        E = E + term
    for _ in range(k):
        E = E @ E
    return E


def _prologue(x, WA, bA, WB, bB, WC, bC):
    x = np.asarray(x, np.float32)
    feat = x.mean(axis=1)
    A = np.tanh(feat @ np.asarray(WA).T + bA).reshape(-1, N, N)
    Bv = np.tanh(feat @ np.asarray(WB).T + bB)
    Cv = np.tanh(feat @ np.asarray(WC).T + bC)
    S = np.triu(A, 1)
    S = S - np.swapaxes(S, -1, -2)
    g = 10.0 ** (-3.0 / SR)
    G = g ** np.asarray(DELAYS, np.float64)
    A_g = np.stack([_expm64(S[b]) for b in range(S.shape[0])])
    A_g = (A_g * G[None, None, :]).astype(np.float32)
    return A_g, Bv.astype(np.float32), Cv.astype(np.float32)


def _core_inputs(A_g4, Bv4, Cv4):
    """Per-core constant tensors in the 128-partition spread layout.

    lhsT_C covers the shallow lines' contribution (rhs = C tile),
    lhsT_S the deep lines' (rhs = gathered S tile). lhsT[k, m]:
    k = rhs partition (source line row), m = out partition.
    """
    lhsT_C = np.zeros((NROW, NROW), np.float32)
    lhsT_4 = np.zeros((NROW, NROW), np.float32)
    lhsT_S = np.zeros((NROW, NROW), np.float32)
    bv = np.zeros((NROW, 1), np.float32)
    for j in range(IPC):
        for i in range(N):
            r = PBASE[i] + j
            dst = lhsT_C if i < 4 else (lhsT_4 if i == 4 else lhsT_S)
            for ip in range(N):
                dst[r, PBASE[ip] + j] = A_g4[j, ip, i]
            dst[r, YBASE + j] = Cv4[j, i]
            bv[r, 0] = Bv4[j, i]
    return lhsT_C, lhsT_4, lhsT_S, bv


def _offsets():
    """offs[r, b-2] = physical flat gather offset for row r, block b.

    Deep-line rows read their [n0-d_i, n0-d_i+500) window; all other
    rows point at their own row's column 0 (zero-init region) so the
    gather stays a single 128-descriptor instruction.
    """
    offs = np.zeros((NROW, NBLK - 2), np.uint32)
    for r in range(NROW):
        offs[r, :] = r * TPAD
    for b in range(2, NBLK):
        n0 = L * b
        for i in DEEP:
            for j in range(IPC):
                r = PBASE[i] + j
                offs[r, b - 2] = r * TPAD + (PAD + n0 - DELAYS[i])
    return offs


def _make_in_maps(A_g, Bv, Cv):
    offs = _offsets()
    in_maps = []
    for k in range(NCORES):
        sl = slice(k * IPC, (k + 1) * IPC)
        lhsT_C, lhsT_4, lhsT_S, bv = _core_inputs(A_g[sl], Bv[sl], Cv[sl])
        in_maps.append({"lhsTC": lhsT_C, "lhsT4": lhsT_4, "lhsTS": lhsT_S,
                        "bv": bv, "offs": offs})
    return in_maps


def _build():
    global _BUILT
    if _BUILT is not None:
        return _BUILT
    import concourse.bacc as bacc
    import concourse.bass as bass
    import concourse.mybir as mybir
    import concourse.tile as tile

    fp32 = mybir.dt.float32
    f32r = mybir.dt.float32r
    u32 = mybir.dt.uint32
    u8 = mybir.dt.uint8
    nc = bacc.Bacc("TRN2", target_bir_lowering=False, debug=False)
    lhsTC_d = nc.dram_tensor("lhsTC", [NROW, NROW], f32r, kind="ExternalInput")
    lhsT4_d = nc.dram_tensor("lhsT4", [NROW, NROW], f32r, kind="ExternalInput")
    lhsTS_d = nc.dram_tensor("lhsTS", [NROW, NROW], f32r, kind="ExternalInput")
    bv_d = nc.dram_tensor("bv", [NROW, 1], f32r, kind="ExternalInput")
    offs_d = nc.dram_tensor("offs", [NROW, NBLK - 2], u32, kind="ExternalInput")
    y_d = nc.dram_tensor("y", [IPC, IR_LEN], f32r, kind="ExternalOutput")
    h_d = nc.dram_tensor("hist", [NROW, TPAD], f32r)

    RW = (RING + 1) * L        # ring cols incl. mirror slot
    SH4 = 3 * L - DELAYS[4]    # line-4 window offset in the ring view
    GLO, GHI = PBASE[NCRIT], PBASE[N - 1] + IPC      # gathered rows 8..56
    WLO, WHI = PBASE[NCRIT], YBASE + IPC             # written rows 8..60

    with tile.TileContext(nc) as tc:
        with tc.tile_pool(name="const", bufs=1) as cpool, \
             tc.tile_pool(name="init", bufs=1) as ipool, \
             tc.tile_pool(name="ring", bufs=1) as rpool, \
             tc.tile_pool(name="sg", bufs=6) as spool, \
             tc.tile_pool(name="cc", bufs=3) as ccpool, \
             tc.tile_pool(name="ps", bufs=8, space="PSUM") as ppool, \
             tc.tile_pool(name="yb", bufs=2) as ypool:
            lhsTC = cpool.tile([NROW, NROW], f32r)
            nc.sync.dma_start(lhsTC[:, :], lhsTC_d[:, :])
            lhsT4 = cpool.tile([NROW, NROW], f32r)
            nc.sync.dma_start(lhsT4[:, :], lhsT4_d[:, :])
            lhsTS = cpool.tile([NROW, NROW], f32r)
            nc.sync.dma_start(lhsTS[:, :], lhsTS_d[:, :])
            offs = cpool.tile([NROW, NBLK - 2], u32)
            nc.sync.dma_start(offs[:, :], offs_d[:, :])

            # DRAM history init: zeros over [0, PAD+2L) with the Bv
            # impulse at col PAD (time 0).
            z = ipool.tile([NROW, PAD + 2 * L], fp32)
            half = (PAD + 2 * L) // 2
            nc.vector.memset(z[:, 0:half], 0.0)
            nc.gpsimd.memset(z[:, half:], 0.0)
            nc.sync.dma_start(z[:, PAD:PAD + 1].bitcast(f32r), bv_d[:, :])
            nc.scalar.dma_start(h_d[:, 0:PAD + 2 * L].bitcast(fp32), z[:, :])

            # Stage ring: slot s holds block b's outputs for b % RING == s;
            # cols [RING*L, RING*L+L) mirror slot 0 so the 2-slot read
            # window stays contiguous across the wrap. Preload: slot 3 :=
            # block -1 (zeros), slot 0 := block 0 (Bv at col 0), slot 1 :=
            # block 1 (zeros).
            ring = rpool.tile([NROW, RW], f32r)
            nc.sync.dma_start(ring[:, :], h_d[:, 0:RW])
            nc.sync.dma_start(ring[:, 0:1], bv_d[:, :])
            nc.vector.tensor_copy(ring[:, RING * L:RING * L + 1],
                                  ring[:, 0:1])
            for cb in range(3):
                ct = ccpool.tile([NROW, L], f32r)
                nc.scalar.dma_start(ct[:, :], h_d[:, 0:L])
            for sb in range(6):
                st = spool.tile([NROW, L], f32r)
                nc.scalar.dma_start(st[:, :], h_d[:, 0:L])

            for b in range(2, NBLK):
                n0 = L * b
                Xb = PAD + n0 - 2 * L   # b-3 frontier (declared coverage)
                S = spool.tile([NROW, L], f32r)
                nc.gpsimd.indirect_dma_start(
                    out=S[:, :], out_offset=None,
                    in_=h_d[0:NROW, 0:Xb],
                    in_offset=bass.IndirectOffsetOnAxis(
                        ap=offs[:, b - 2:b - 1], axis=1),
                )

                # ring view holding blocks b-3, b-2 contiguously
                s3 = (b - 3) % RING
                w0 = s3 * L
                rw = ring[:, w0:w0 + 2 * L]

                # shallow-line movers: C[line rows, :] <- ring window at
                # the line's shift (cols 3L-d_i .. 3L-d_i+500)
                C = ccpool.tile([NROW, L], f32r)
                for i in range(4):
                    q = PBASE[i]
                    sh = 3 * L - DELAYS[i]
                    nc.vector.tensor_copy(C[q:q + IPC, :],
                                          rw[q:q + IPC, sh:sh + L])
                ps = ppool.tile([NROW, L], fp32)
                nc.tensor.matmul(ps[:, :], lhsTC[:, :], C[:, :],
                                 start=True, stop=False)
                nc.tensor.matmul(ps[:, :], lhsT4[:, :],
                                 rw[:, SH4:SH4 + L],
                                 start=False, stop=False)
                nc.tensor.matmul(ps[:, :], lhsTS[:, :], S[:, :],
                                 start=False, stop=True)

                # drain to ring slot (ACT), mirror slot 0 for the wrap
                slot = b % RING
                dst = ring[:, slot * L:slot * L + L]
                nc.scalar.copy(dst, ps[:, :])
                if slot == 0:
                    nc.vector.tensor_copy(
                        ring[:, RING * L:RING * L + L], ring[:, 0:L])

                weng = nc.sync if b % 2 == 0 else nc.scalar
                weng.dma_start(h_d[WLO:WHI, PAD + n0:PAD + n0 + L],
                               dst[WLO:WHI, :])

                # y extraction, interleaved with compute (as in v3)
                CH = 12000
                if b >= 25 and (b - 25) % 24 == 0 and (k := (b - 25) // 24) < 3:
                    yb = ypool.tile([IPC, CH], f32r)
                    nc.scalar.dma_start(
                        yb[:, :],
                        h_d[YBASE:YBASE + IPC, PAD + k * CH:PAD + (k + 1) * CH])
                    nc.scalar.dma_start(y_d[:, k * CH:(k + 1) * CH], yb[:, :])
                if b >= NBLK - 2:
                    nc.sync.dma_start(
                        y_d[:, n0:n0 + L], dst[YBASE:YBASE + IPC, :])

            yb = ypool.tile([IPC, CH], f32r)
            span = IR_LEN - L * 2 - 3 * CH
            nc.scalar.dma_start(
                yb[:, 0:span],
                h_d[YBASE:YBASE + IPC, PAD + 3 * CH:PAD + 3 * CH + span])
            nc.scalar.dma_start(y_d[:, 3 * CH:3 * CH + span], yb[:, 0:span])
    nc.compile()
    _BUILT = nc
    return nc


def kernel(x, WA, bA, WB, bB, WC, bC):
    from concourse import bass_utils

    A_g, Bv, Cv = _prologue(x, WA, bA, WB, bB, WC, bC)
    in_maps = _make_in_maps(A_g, Bv, Cv)
    nc = _build()
    res = bass_utils.run_bass_kernel_spmd(nc, in_maps, core_ids=list(range(NCORES)))
    y = np.concatenate([res.results[k]["y"] for k in range(NCORES)], axis=0)
    return y[:, None, :].astype(np.float32)


# revision 8
# speedup vs baseline: 1.4347x; 1.0786x over previous
"""DiffFDN Trainium2 kernel, v5: SBUF ring fast-path + slack DMA gather.

Per core (4 items), the 48000-step FDN scan runs as 94 blocks of L=500
timesteps. The serial dependency (min delay 1009) only couples block b
to blocks <= b-2, and the per-block work is one [128x128]^T @ [128x500]
matmul pair.

Key structure (vs the old all-DMA pipeline):
- Deep lines (d >= 1543, lines 5-15): windows come from blocks <= b-3,
  fetched by ONE SWDGE indirect gather per block whose declared in_ AP
  prefix ends at the b-3 frontier -- so it waits only on write(b-3),
  giving the DMA round trip a full extra link of slack off the critical
  chain.
- Shallow lines (d < 1500, lines 0-4): their windows touch block b-2,
  too fresh for DRAM. They are assembled into a separate C tile from an
  SBUF stage ring (drain targets) by 4 plain DVE copies at quadrant
  partition bases (0/32/64/96; engines require 32-aligned partition
  bases) plus one predicated copy for line 4. The serial chain is then
  drain(b-2) -> movers(b) -> mm_C(b): engine ops only, no DMA.
- Row layout spreads the 64 (line,item) rows across 128 partitions so
  each shallow line sits at its own 32-aligned base; lhsT rows for
  unused partitions are zero, which makes the extra partitions free in
  the matmul (cost scales only with the 500-wide free dim).
"""

import numpy as np

SR = 48000
IR_LEN = 48000
DELAYS = [1009, 1123, 1231, 1321, 1433, 1543, 1657, 1777, 1879, 1987,
          2081, 2179, 2287, 2383, 2503, 2617]
N = 16
FEAT = 256
BATCH = 32
NCORES = 8
IPC = BATCH // NCORES
L = 500
PAD = 2620                     # zero padding before t=0 (>= max delay)
TPAD = PAD + IR_LEN + 500
NBLK = IR_LEN // L             # 96; blocks 0,1 are part of the init image
NROW = 128
RING = 4                       # stage-ring slots (plus one mirror slot)

# partition base of each line's 4 item-rows; shallow lines 0-3 take the
# 32-aligned quadrant bases so plain DVE copies can address them.
PBASE = [0, 32, 64, 96, 4, 8, 12, 16, 20, 24, 28, 36, 40, 44, 48, 52]
YBASE = 56                     # y output rows (4)
NCRIT = 5                      # lines 0-4: d < 3L, SBUF fast path
DEEP = list(range(NCRIT, N))   # lines via DRAM gather

_BUILT = None


def _expm64(M):
    M = M.astype(np.float64)
    nrm = np.linalg.norm(M, ord=np.inf)
    k = max(0, int(np.ceil(np.log2(max(nrm, 1e-30)))) + 2)
    Ms = M / (2.0 ** k)
    E = np.eye(M.shape[0]) + Ms
    term = Ms.copy()
    for i in range(2, 18):
        term = term @ Ms / i
        E = E + term
    for _ in range(k):
        E = E @ E
    return E


def _prologue(x, WA, bA, WB, bB, WC, bC):
    x = np.asarray(x, np.float32)
    feat = x.mean(axis=1)
    A = np.tanh(feat @ np.asarray(WA).T + bA).reshape(-1, N, N)
    Bv = np.tanh(feat @ np.asarray(WB).T + bB)
    Cv = np.tanh(feat @ np.asarray(WC).T + bC)
    S = np.triu(A, 1)
    S = S - np.swapaxes(S, -1, -2)
    g = 10.0 ** (-3.0 / SR)
    G = g ** np.asarray(DELAYS, np.float64)
    A_g = np.stack([_expm64(S[b]) for b in range(S.shape[0])])
    A_g = (A_g * G[None, None, :]).astype(np.float32)
    return A_g, Bv.astype(np.float32), Cv.astype(np.float32)


def _core_inputs(A_g4, Bv4, Cv4):
    """Per-core constant tensors in the 128-partition spread layout.

    lhsT_C covers the shallow lines' contribution (rhs = C tile),
    lhsT_S the deep lines' (rhs = gathered S tile). lhsT[k, m]:
    k = rhs partition (source line row), m = out partition.
    """
    lhsT_C = np.zeros((NROW, NROW), np.float32)
    lhsT_4 = np.zeros((NROW, NROW), np.float32)
    lhsT_S = np.zeros((NROW, NROW), np.float32)
    bv = np.zeros((NROW, 1), np.float32)
    for j in range(IPC):
        for i in range(N):
            r = PBASE[i] + j
            dst = lhsT_C if i < 4 else (lhsT_4 if i == 4 else lhsT_S)
            for ip in range(N):
                dst[r, PBASE[ip] + j] = A_g4[j, ip, i]
            dst[r, YBASE + j] = Cv4[j, i]
            bv[r, 0] = Bv4[j, i]
    return lhsT_C, lhsT_4, lhsT_S, bv


def _offsets():
    """offs[r, b-2] = physical flat gather offset for row r, block b.

    Deep-line rows read their [n0-d_i, n0-d_i+500) window; all other
    rows point at their own row's column 0 (zero-init region) so the
    gather stays a single 128-descriptor instruction.
    """
    offs = np.zeros((NROW, NBLK - 2), np.uint32)
    for r in range(NROW):
        offs[r, :] = r * TPAD
    for b in range(2, NBLK):
        n0 = L * b
        for i in DEEP:
            for j in range(IPC):
                r = PBASE[i] + j
                offs[r, b - 2] = r * TPAD + (PAD + n0 - DELAYS[i])
    return offs


def _make_in_maps(A_g, Bv, Cv):
    offs = _offsets()
    in_maps = []
    for k in range(NCORES):
        sl = slice(k * IPC, (k + 1) * IPC)
        lhsT_C, lhsT_4, lhsT_S, bv = _core_inputs(A_g[sl], Bv[sl], Cv[sl])
        in_maps.append({"lhsTC": lhsT_C, "lhsT4": lhsT_4, "lhsTS": lhsT_S,
                        "bv": bv, "offs": offs})
    return in_maps


def _build():
    global _BUILT
    if _BUILT is not None:
        return _BUILT
    import concourse.bacc as bacc
    import concourse.bass as bass
    import concourse.mybir as mybir
    import concourse.tile as tile

    fp32 = mybir.dt.float32
    f32r = mybir.dt.float32r
    u32 = mybir.dt.uint32
    u8 = mybir.dt.uint8
    nc = bacc.Bacc("TRN2", target_bir_lowering=False, debug=False)
    lhsTC_d = nc.dram_tensor("lhsTC", [NROW, NROW], f32r, kind="ExternalInput")
    lhsT4_d = nc.dram_tensor("lhsT4", [NROW, NROW], f32r, kind="ExternalInput")
    lhsTS_d = nc.dram_tensor("lhsTS", [NROW, NROW], f32r, kind="ExternalInput")
    bv_d = nc.dram_tensor("bv", [NROW, 1], f32r, kind="ExternalInput")
    offs_d = nc.dram_tensor("offs", [NROW, NBLK - 2], u32, kind="ExternalInput")
    y_d = nc.dram_tensor("y", [IPC, IR_LEN], f32r, kind="ExternalOutput")
    h_d = nc.dram_tensor("hist", [NROW, TPAD], f32r)

    RW = (RING + 1) * L        # ring cols incl. mirror slot
    SH4 = 3 * L - DELAYS[4]    # line-4 window offset in the ring view
    GLO, GHI = PBASE[NCRIT], PBASE[N - 1] + IPC      # gathered rows 8..56
    WLO, WHI = PBASE[NCRIT], YBASE + IPC             # written rows 8..60

    with tile.TileContext(nc) as tc:
        with tc.tile_pool(name="const", bufs=1) as cpool, \
             tc.tile_pool(name="init", bufs=1) as ipool, \
             tc.tile_pool(name="ring", bufs=1) as rpool, \
             tc.tile_pool(name="sg", bufs=6) as spool, \
             tc.tile_pool(name="cc", bufs=3) as ccpool, \
             tc.tile_pool(name="ps", bufs=8, space="PSUM") as ppool, \
             tc.tile_pool(name="yb", bufs=2) as ypool:
            lhsTC = cpool.tile([NROW, NROW], f32r)
            nc.sync.dma_start(lhsTC[:, :], lhsTC_d[:, :])
            lhsT4 = cpool.tile([NROW, NROW], f32r)
            nc.sync.dma_start(lhsT4[:, :], lhsT4_d[:, :])
            lhsTS = cpool.tile([NROW, NROW], f32r)
            nc.sync.dma_start(lhsTS[:, :], lhsTS_d[:, :])
            offs = cpool.tile([NROW, NBLK - 2], u32)
            nc.sync.dma_start(offs[:, :], offs_d[:, :])

            # DRAM history init: zeros over [0, PAD+2L) with the Bv
            # impulse at col PAD (time 0).
            z = ipool.tile([NROW, PAD + 2 * L], fp32)
            half = (PAD + 2 * L) // 2
            nc.vector.memset(z[:, 0:half], 0.0)
            nc.gpsimd.memset(z[:, half:], 0.0)
            nc.sync.dma_start(z[:, PAD:PAD + 1].bitcast(f32r), bv_d[:, :])
            nc.scalar.dma_start(h_d[:, 0:PAD + 2 * L].bitcast(fp32), z[:, :])

            # Stage ring: slot s holds block b's outputs for b % RING == s;
            # cols [RING*L, RING*L+L) mirror slot 0 so the 2-slot read
            # window stays contiguous across the wrap. Preload: slot 3 :=
            # block -1 (zeros), slot 0 := block 0 (Bv at col 0), slot 1 :=
            # block 1 (zeros).
            ring = rpool.tile([NROW, RW], f32r)
            nc.sync.dma_start(ring[:, :], h_d[:, 0:RW])
            nc.sync.dma_start(ring[:, 0:1], bv_d[:, :])
            nc.vector.tensor_copy(ring[:, RING * L:RING * L + 1],
                                  ring[:, 0:1])
            for cb in range(3):
                ct = ccpool.tile([NROW, L], f32r)
                nc.scalar.dma_start(ct[:, :], h_d[:, 0:L])
            for sb in range(6):
                st = spool.tile([NROW, L], f32r)
                nc.scalar.dma_start(st[:, :], h_d[:, 0:L])

            for b in range(2, NBLK):
                n0 = L * b
                Xb = PAD + n0 - 2 * L   # b-3 frontier (declared coverage)
                S = spool.tile([NROW, L], f32r)
                nc.gpsimd.indirect_dma_start(
                    out=S[0:GHI, :], out_offset=None,
                    in_=h_d[0:NROW, 0:Xb],
                    in_offset=bass.IndirectOffsetOnAxis(
                        ap=offs[0:GHI, b - 2:b - 1], axis=1),
                )

                # ring view holding blocks b-3, b-2 contiguously
                s3 = (b - 3) % RING
                w0 = s3 * L
                rw = ring[:, w0:w0 + 2 * L]

                # shallow-line movers: C[line rows, :] <- ring window at
                # the line's shift (cols 3L-d_i .. 3L-d_i+500)
                C = ccpool.tile([NROW, L], f32r)
                for i in range(4):
                    q = PBASE[i]
                    sh = 3 * L - DELAYS[i]
                    nc.vector.tensor_copy(C[q:q + IPC, :],
                                          rw[q:q + IPC, sh:sh + L])
                ps = ppool.tile([NROW, L], fp32)
                nc.tensor.matmul(ps[:, :], lhsTC[:, :], C[:, :],
                                 start=True, stop=False)
                nc.tensor.matmul(ps[:, :], lhsT4[:, :],
                                 rw[:, SH4:SH4 + L],
                                 start=False, stop=False)
                nc.tensor.matmul(ps[:, :], lhsTS[:, :], S[:, :],
                                 start=False, stop=True)

                # drain to ring slot (ACT), mirror slot 0 for the wrap
                slot = b % RING
                dst = ring[:, slot * L:slot * L + L]
                nc.scalar.copy(dst, ps[:, :])
                if slot == 0:
                    # wrap margin: second PSUM drain in parallel on DVE
                    nc.vector.tensor_copy(
                        ring[:, RING * L:RING * L + L], ps[:, :])

                weng = nc.sync if b % 2 == 0 else nc.scalar
                weng.dma_start(h_d[WLO:WHI, PAD + n0:PAD + n0 + L],
                               dst[WLO:WHI, :])

                # y extraction, interleaved with compute (as in v3)
                CH = 12000
                if b >= 25 and (b - 25) % 24 == 0 and (k := (b - 25) // 24) < 3:
                    yb = ypool.tile([IPC, CH], f32r)
                    nc.scalar.dma_start(
                        yb[:, :],
                        h_d[YBASE:YBASE + IPC, PAD + k * CH:PAD + (k + 1) * CH])
                    nc.scalar.dma_start(y_d[:, k * CH:(k + 1) * CH], yb[:, :])
                if b >= NBLK - 2:
                    nc.sync.dma_start(
                        y_d[:, n0:n0 + L], dst[YBASE:YBASE + IPC, :])

            yb = ypool.tile([IPC, CH], f32r)
            span = IR_LEN - L * 2 - 3 * CH
            nc.scalar.dma_start(
                yb[:, 0:span],
                h_d[YBASE:YBASE + IPC, PAD + 3 * CH:PAD + 3 * CH + span])
            nc.scalar.dma_start(y_d[:, 3 * CH:3 * CH + span], yb[:, 0:span])
    nc.compile()
    _BUILT = nc
    return nc


def kernel(x, WA, bA, WB, bB, WC, bC):
    from concourse import bass_utils

    A_g, Bv, Cv = _prologue(x, WA, bA, WB, bB, WC, bC)
    in_maps = _make_in_maps(A_g, Bv, Cv)
    nc = _build()
    res = bass_utils.run_bass_kernel_spmd(nc, in_maps, core_ids=list(range(NCORES)))
    y = np.concatenate([res.results[k]["y"] for k in range(NCORES)], axis=0)
    return y[:, None, :].astype(np.float32)
